# revision 25
# baseline (speedup 1.0000x reference)
"""TRN2 Bass kernel for nn_FFTMLP_86904368267649.

Reference math: energies[b,o] = sum_f xr[b,f]*w_r[o,f] + xi[b,f]*w_i[o,f]
with w_r = fr+fi, w_i = fr-fi, x: [B, 2, F] fp32, filters: [O, F] fp32.

Structure exploited (two levels):
 1. Filter periodicity (period O=1024 in f): the F=2049 contraction folds
    to T=1024 per channel: xr'[t] = xr[t] + xr[t+1024] (+ wrap into t=0).
 2. DFT reflection symmetry: with u = xr'+xi', v = xr'-xi' and
    C[t,o] = 0.02*cos(2*pi*o*t/1024), S[t,o] = 0.02*sin(...),
      energies[:, o]      = (u@C + v@S)[:, o]            o = 0..511
      energies[:, 1024-o] = (u@C - v@S)[:, o]            o = 1..511
      energies[:, 512]    = u @ C[:, 512]  (extra 1-col matmul into the
                            sin bank's col 0, whose sin weights are 0)
    The device ships S1 = E+ + E-, S2 = E+ - E- (bf16); the host
    unscrambles (reversal + col-0/512 recombination) during the gather.
    This halves PE work vs the direct [2048k x 1024o] matmul.

Everything on the wire is bf16 (PSUM accumulates f32): ~23 MB/core of
HBM traffic vs 50 MB for the fp32 direct kernel.

Sharding: data-parallel over batch, 2048 rows per core across 8 cores.
x ships k-major, partition-major, quartered over b: DRAM row
(bq, cls, p) holds [kt 0..7][512 b-cols] for partition p of row-class
cls in (xr_lo, xr_hi, xi_lo, xi_hi) -- 8 KB contiguous per partition,
one SWDGE transfer per (quarter, class, kt-half). DVE folds in place
(cls_lo += cls_hi) and forms u/v with fully contiguous APs (u,v are
bq-major so each quarter-half write is flat). ACT drains E+, DVE
assembles S1/S2, and PSUM holds exactly one quarter wave (2 passes x
4 b-subtiles = 8 banks) with k-inner accumulation; kt0-3 matmuls of a
wave start once the first kt-half of that quarter is folded. Dummy
matmuls during the initial fill keep the PE p-state ramped.
"""

import sys

if "/opt/trn_rl_repo" not in sys.path:
    sys.path.insert(0, "/opt/trn_rl_repo")

import numpy as np
import ml_dtypes

import concourse.bass as bass
import concourse.mybir as mybir
import concourse.tile as tile
from concourse import bacc
from concourse.bass_utils import run_bass_kernel_spmd

BF16NP = ml_dtypes.bfloat16
B, O, F, T = 16384, 1024, 2049, 1024
NCORES = 8
BS = B // NCORES          # 2048 batch rows per core
KT = T // 128             # 8 k-tiles over the folded t contraction
KH = 4                    # kt-hop granularity for fold pipelining
OC = 512                  # o-columns per pass (= one PSUM bank of f32)
BQ = 4                    # b-quarters (wave granularity)
QW = BS // BQ             # 512 b-cols per quarter
BSUB = 4                  # 128-row b-subtiles per quarter
NWARM = 32                # warmup matmuls bridging the PE to first fold
F32 = mybir.dt.float32
BF16 = mybir.dt.bfloat16

_CACHE = {}
LAST_RESULTS = None


def _build():
    nc = bacc.Bacc("TRN2", target_bir_lowering=False, debug=False,
                   num_devices=NCORES)

    # row (bq, cls, p) = [kt][b-seg]; cls in (xr_lo, xr_hi, xi_lo, xi_hi)
    xt_dram = nc.dram_tensor("xT", [BQ * 4 * 128, KT * QW], BF16,
                             kind="ExternalInput")
    wr_dram = nc.dram_tensor("wrap", [2, BS], BF16, kind="ExternalInput")
    # w rows = t, cols = [C (o=0..511) | S (o=0..511, col0 zeroed)]
    w_dram = nc.dram_tensor("w", [T, 2 * OC], BF16, kind="ExternalInput")
    # per-partition extra column: 0.02*cos(pi*p) for the o=512 output
    g_dram = nc.dram_tensor("g", [128, 1], BF16, kind="ExternalInput")
    # out rows = b, cols = [S1 | S2]
    out_dram = nc.dram_tensor("out", [BS, 2 * OC], BF16, kind="ExternalOutput")

    with tile.TileContext(nc) as tc:
        with (
            tc.tile_pool(name="const", bufs=1) as const,
            tc.tile_pool(name="raw", bufs=3) as rawp,
            tc.tile_pool(name="ecp", bufs=6) as ecp,
            tc.tile_pool(name="outp", bufs=5) as outp,
            tc.tile_pool(name="psum", bufs=8, space="PSUM") as psum,
        ):
            xta = xt_dram.ap().rearrange("r (k s) -> r k s", k=KT)
            w_ap = w_dram.ap().rearrange("(kt p) o -> p kt o", p=128)
            out_ap = out_dram.ap()

            gt = const.tile([128, 1], BF16)
            nc.scalar.dma_start(gt[:], g_dram.ap())
            wt = const.tile([128, KT, 2 * OC], BF16)
            for kt in range(KT):
                nc.sync.dma_start(wt[:, kt], w_ap[:, kt])
            wrapt = const.tile([1, 2, BS], BF16)
            nc.scalar.dma_start(wrapt[0:1, 0], wr_dram.ap()[0:1, :])
            nc.scalar.dma_start(wrapt[0:1, 1], wr_dram.ap()[1:2, :])

            # u,v are bq-major: [128, bq, kt, b-seg]
            u = const.tile([128, BQ, KT, QW], BF16)
            v = const.tile([128, BQ, KT, QW], BF16)

            # PE p-state warmup: garbage matmuls into a scratch bank while
            # the first quarter streams in (bank is reset by its real wave)
            warm = psum.tile([128, OC], F32, tag="ps", name="warm")
            for i in range(NWARM):
                nc.tensor.matmul(warm[:], wt[:, 0, 0:128], wt[:, 0, :OC],
                                 start=True, stop=True, skip_group_check=True)

            # phase 1: stream quarters (kt-half granular), fold to u/v
            for bq in range(BQ):
                qs = bq * QW
                r = [rawp.tile([128, KT, QW], BF16, tag=f"r{j}",
                               name=f"r{j}_{bq}") for j in range(4)]
                for h in range(KT // KH):
                    hs = slice(KH * h, KH * h + KH)
                    for j in range(4):
                        r0 = (bq * 4 + j) * 128
                        nc.gpsimd.dma_start(r[j][:, hs],
                                            xta[r0:r0 + 128, hs])
                    # in-place fold: lo += hi
                    nc.vector.tensor_add(out=r[0][:, hs], in0=r[0][:, hs],
                                         in1=r[1][:, hs])
                    nc.vector.tensor_add(out=r[2][:, hs], in0=r[2][:, hs],
                                         in1=r[3][:, hs])
                    if h == 0:
                        # wrap rows (f=2048) into t=0 (kt=0, partition 0)
                        nc.vector.tensor_add(
                            out=r[0][0:1, 0], in0=r[0][0:1, 0],
                            in1=wrapt[0:1, 0, qs:qs + QW])
                        nc.vector.tensor_add(
                            out=r[2][0:1, 0], in0=r[2][0:1, 0],
                            in1=wrapt[0:1, 1, qs:qs + QW])
                    nc.vector.tensor_add(out=u[:, bq, hs], in0=r[0][:, hs],
                                         in1=r[2][:, hs])
                    nc.vector.tensor_sub(out=v[:, bq, hs], in0=r[0][:, hs],
                                         in1=r[2][:, hs])

            # phase 2: per-quarter matmul waves + assembly
            for bq in range(BQ):
                ps_p = [psum.tile([128, OC], F32, tag="ps",
                                  name=f"psp{bq}_{s}") for s in range(BSUB)]
                ps_m = [psum.tile([128, OC], F32, tag="ps",
                                  name=f"psm{bq}_{s}") for s in range(BSUB)]
                # wave 0 is fold-paced: kt-outer lets all banks progress as
                # kt-halves land. Later waves run on resident u,v: bsub-outer
                # finishes each bank early so its assembly + out-DMA overlap
                # the remaining matmuls instead of piling up at wave end.
                if bq == 0:
                    order = [(kt, s) for kt in range(KT) for s in range(BSUB)]
                else:
                    order = [(kt, s) for s in range(BSUB) for kt in range(KT)]
                for kt, s in order:
                    st, sp = (kt == 0), (kt == KT - 1)
                    b0 = s * 128
                    lv = v[:, bq, kt, b0:b0 + 128]
                    lu = u[:, bq, kt, b0:b0 + 128]
                    nc.tensor.matmul(ps_m[s][:], lv, wt[:, kt, OC:],
                                     start=st, stop=False,
                                     skip_group_check=True)
                    nc.tensor.matmul(ps_p[s][:], lu, wt[:, kt, :OC],
                                     start=st, stop=sp)
                    # o=512 column rides on the sin bank's col 0
                    nc.tensor.matmul(ps_m[s][:, 0:1], lu, gt[:, 0:1],
                                     start=False, stop=sp,
                                     skip_group_check=True)
                for s in range(BSUB):
                    b0 = bq * QW + s * 128
                    ec = ecp.tile([128, OC], F32, tag="ec", name=f"ec{bq}_{s}")
                    nc.scalar.copy(ec[:], ps_p[s][:])
                    ot = outp.tile([128, 2, OC], BF16, tag="out",
                                   name=f"ot{bq}_{s}")
                    nc.vector.tensor_add(out=ot[:, 0], in0=ec[:],
                                         in1=ps_m[s][:])
                    nc.vector.tensor_sub(out=ot[:, 1], in0=ec[:],
                                         in1=ps_m[s][:])
                    nc.sync.dma_start(out_ap[b0:b0 + 128, :], ot[:])

    nc.compile()
    return nc


def kernel(x, filters_real, filters_imag):
    global LAST_RESULTS
    x = np.asarray(x, dtype=np.float32)
    fr = np.asarray(filters_real, dtype=np.float32)
    fi = np.asarray(filters_imag, dtype=np.float32)

    # weights: C = (w_r+w_i)/2 = 0.02cos, S = (w_r-w_i)/2 = 0.02sin over
    # the first period, transposed to [t, o]; o = 0..511 plus the o=512
    # cos column served by g (and sin col 0, identically 0, zeroed).
    w_r = fr + fi                           # [O, F]
    w_i = fr - fi
    cfull = 0.5 * (w_r[:, :T] + w_i[:, :T])   # [O, T] = 0.02 cos
    sfull = 0.5 * (w_r[:, :T] - w_i[:, :T])   # [O, T] = 0.02 sin
    w_np = np.empty((T, 2 * OC), np.float32)
    w_np[:, :OC] = cfull[:OC].T
    w_np[:, OC:] = sfull[:OC].T
    w_np[:, OC] = 0.0
    w_np = w_np.astype(BF16NP)
    g_np = np.ascontiguousarray(cfull[OC, :128][:, None]).astype(BF16NP)

    if "nc" not in _CACHE:
        _CACHE["nc"] = _build()
    nc = _CACHE["nc"]

    xbf = x.astype(BF16NP)                  # [B, 2, F]
    from concurrent.futures import ThreadPoolExecutor

    def _shard(c):
        xs = xbf[c * BS:(c + 1) * BS]       # [2048, 2, 2049]
        xt = np.empty((BQ, 4, 128, KT, QW), BF16NP)
        for ch in range(2):
            xct = np.ascontiguousarray(xs[:, ch, :2 * T].T)  # [2048t, 2048b]
            # [kt, p, bq, bs] -> [bq, p, kt, bs]
            lo = xct[:T].reshape(KT, 128, BQ, QW).transpose(2, 1, 0, 3)
            hi = xct[T:].reshape(KT, 128, BQ, QW).transpose(2, 1, 0, 3)
            xt[:, 2 * ch] = lo
            xt[:, 2 * ch + 1] = hi
        wrap = np.ascontiguousarray(xs[:, :, 2 * T].T)       # [2, 2048]
        return xt.reshape(BQ * 4 * 128, KT * QW), wrap

    with ThreadPoolExecutor(NCORES) as ex:
        shards = list(ex.map(_shard, range(NCORES)))
    in_maps = [{"xT": shards[c][0], "wrap": shards[c][1],
                "w": w_np, "g": g_np} for c in range(NCORES)]

    import os
    trace = bool(os.environ.get("BASS_TRACE"))
    if trace:
        try:
            import antenv.axon_hooks  # noqa: F401  (shim from test.py)
        except ImportError:
            trace = False
            os.environ["BASS_NEVER_TRACE"] = "1"
    res = run_bass_kernel_spmd(nc, in_maps, list(range(NCORES)), trace=trace)
    LAST_RESULTS = res

    out = np.empty((B, O), np.float32)

    def _gather(c):
        sc = np.asarray(res.results[c]["out"]).astype(np.float32)
        s1, s2 = sc[:, :OC], sc[:, OC:]
        oc = out[c * BS:(c + 1) * BS]
        oc[:, 0] = 0.5 * (s1[:, 0] + s2[:, 0])
        oc[:, 1:OC] = s1[:, 1:OC]
        oc[:, OC] = 0.5 * (s1[:, 0] - s2[:, 0])
        oc[:, OC + 1:] = s2[:, OC - 1:0:-1]

    with ThreadPoolExecutor(NCORES) as ex:
        list(ex.map(_gather, range(NCORES)))
    return out


# revision 28
# speedup vs baseline: 1.0317x; 1.0317x over previous
"""TRN2 Bass kernel for nn_FFTMLP_86904368267649.

Reference math: energies[b,o] = sum_f xr[b,f]*w_r[o,f] + xi[b,f]*w_i[o,f]
with w_r = fr+fi, w_i = fr-fi, x: [B, 2, F] fp32, filters: [O, F] fp32.

Structure exploited (two levels):
 1. Filter periodicity (period O=1024 in f): the F=2049 contraction folds
    to T=1024 per channel: xr'[t] = xr[t] + xr[t+1024] (+ wrap into t=0).
 2. DFT reflection symmetry: with u = xr'+xi', v = xr'-xi' and
    C[t,o] = 0.02*cos(2*pi*o*t/1024), S[t,o] = 0.02*sin(...),
      energies[:, o]      = (u@C + v@S)[:, o]            o = 0..511
      energies[:, 1024-o] = (u@C - v@S)[:, o]            o = 1..511
      energies[:, 512]    = u @ C[:, 512]  (extra 1-col matmul into the
                            sin bank's col 0, whose sin weights are 0)
    The device ships S1 = E+ + E-, S2 = E+ - E- (bf16); the host
    unscrambles (reversal + col-0/512 recombination) during the gather.
    This halves PE work vs the direct [2048k x 1024o] matmul.

Everything on the wire is bf16 (PSUM accumulates f32): ~23 MB/core of
HBM traffic vs 50 MB for the fp32 direct kernel.

Sharding: data-parallel over batch, 2048 rows per core across 8 cores.
x ships k-major, partition-major, quartered over b: DRAM row
(bq, cls, p) holds [kt 0..7][512 b-cols] for partition p of row-class
cls in (xr_lo, xr_hi, xi_lo, xi_hi) -- 8 KB contiguous per partition,
one SWDGE transfer per (quarter, class, kt-half). DVE folds in place
(cls_lo += cls_hi) and forms u/v with fully contiguous APs (u,v are
bq-major so each quarter-half write is flat). ACT drains E+, DVE
assembles S1/S2, and PSUM holds exactly one quarter wave (2 passes x
4 b-subtiles = 8 banks) with k-inner accumulation; kt0-3 matmuls of a
wave start once the first kt-half of that quarter is folded. Dummy
matmuls during the initial fill keep the PE p-state ramped.
"""

import sys

if "/opt/trn_rl_repo" not in sys.path:
    sys.path.insert(0, "/opt/trn_rl_repo")

import numpy as np
import ml_dtypes

import concourse.bass as bass
import concourse.mybir as mybir
import concourse.tile as tile
from concourse import bacc
from concourse.bass_utils import run_bass_kernel_spmd

BF16NP = ml_dtypes.bfloat16
B, O, F, T = 16384, 1024, 2049, 1024
NCORES = 8
BS = B // NCORES          # 2048 batch rows per core
KT = T // 128             # 8 k-tiles over the folded t contraction
KH = 4                    # kt-hop granularity for fold pipelining
OC = 512                  # o-columns per pass (= one PSUM bank of f32)
BQ = 4                    # b-quarters (wave granularity)
QW = BS // BQ             # 512 b-cols per quarter
BSUB = 4                  # 128-row b-subtiles per quarter
NWARM = 48                # warmup matmuls to hold PE p-state during fill
F32 = mybir.dt.float32
BF16 = mybir.dt.bfloat16

_CACHE = {}
LAST_RESULTS = None


def _build():
    nc = bacc.Bacc("TRN2", target_bir_lowering=False, debug=False,
                   num_devices=NCORES)

    # row (bq, cls, p) = [kt][b-seg]; cls in (xr_lo, xr_hi, xi_lo, xi_hi)
    xt_dram = nc.dram_tensor("xT", [BQ * 4 * 128, KT * QW], BF16,
                             kind="ExternalInput")
    wr_dram = nc.dram_tensor("wrap", [2, BS], BF16, kind="ExternalInput")
    # w rows = t, cols = [C (o=0..511) | S (o=0..511, col0 zeroed)]
    w_dram = nc.dram_tensor("w", [T, 2 * OC], BF16, kind="ExternalInput")
    # per-partition extra column: 0.02*cos(pi*p) for the o=512 output
    g_dram = nc.dram_tensor("g", [128, 1], BF16, kind="ExternalInput")
    # out rows = b, cols = [S1 | S2]
    out_dram = nc.dram_tensor("out", [BS, 2 * OC], BF16, kind="ExternalOutput")

    with tile.TileContext(nc) as tc:
        with (
            tc.tile_pool(name="const", bufs=1) as const,
            tc.tile_pool(name="raw", bufs=2) as rawp,
            tc.tile_pool(name="ecp", bufs=6) as ecp,
            tc.tile_pool(name="outp", bufs=5) as outp,
            tc.tile_pool(name="psum", bufs=8, space="PSUM") as psum,
        ):
            xta = xt_dram.ap().rearrange("r (k s) -> r k s", k=KT)
            w_ap = w_dram.ap().rearrange("(kt p) o -> p kt o", p=128)
            out_ap = out_dram.ap()

            gt = const.tile([128, 1], BF16)
            nc.scalar.dma_start(gt[:], g_dram.ap())
            wt = const.tile([128, KT, 2 * OC], BF16)
            for kt in range(KT):
                nc.sync.dma_start(wt[:, kt], w_ap[:, kt])
            wrapt = const.tile([1, 2, BS], BF16)
            nc.scalar.dma_start(wrapt[0:1, 0], wr_dram.ap()[0:1, :])
            nc.scalar.dma_start(wrapt[0:1, 1], wr_dram.ap()[1:2, :])

            # u,v are bq-major: [128, bq, kt, b-seg]
            u = const.tile([128, BQ, KT, QW], BF16)
            v = const.tile([128, BQ, KT, QW], BF16)

            # PE p-state warmup: garbage matmuls into a scratch bank while
            # the first quarter streams in (bank is reset by its real wave)
            warm = psum.tile([128, OC], F32, tag="ps", name="warm")
            for i in range(NWARM):
                nc.tensor.matmul(warm[:], wt[:, 0, 0:128], wt[:, 0, :OC],
                                 start=True, stop=True, skip_group_check=True)

            # phase 1: stream quarters (kt-half granular), fold to u/v
            for bq in range(BQ):
                qs = bq * QW
                r = [rawp.tile([128, KT, QW], BF16, tag=f"r{j}",
                               name=f"r{j}_{bq}") for j in range(4)]
                for h in range(KT // KH):
                    hs = slice(KH * h, KH * h + KH)
                    for j in range(4):
                        r0 = (bq * 4 + j) * 128
                        nc.gpsimd.dma_start(r[j][:, hs],
                                            xta[r0:r0 + 128, hs])
                    # in-place fold: lo += hi
                    nc.vector.tensor_add(out=r[0][:, hs], in0=r[0][:, hs],
                                         in1=r[1][:, hs])
                    nc.vector.tensor_add(out=r[2][:, hs], in0=r[2][:, hs],
                                         in1=r[3][:, hs])
                    if h == 0:
                        # wrap rows (f=2048) into t=0 (kt=0, partition 0)
                        nc.vector.tensor_add(
                            out=r[0][0:1, 0], in0=r[0][0:1, 0],
                            in1=wrapt[0:1, 0, qs:qs + QW])
                        nc.vector.tensor_add(
                            out=r[2][0:1, 0], in0=r[2][0:1, 0],
                            in1=wrapt[0:1, 1, qs:qs + QW])
                    nc.vector.tensor_add(out=u[:, bq, hs], in0=r[0][:, hs],
                                         in1=r[2][:, hs])
                    nc.vector.tensor_sub(out=v[:, bq, hs], in0=r[0][:, hs],
                                         in1=r[2][:, hs])

            # phase 2: per-quarter matmul waves + assembly
            for bq in range(BQ):
                ps_p = [psum.tile([128, OC], F32, tag="ps",
                                  name=f"psp{bq}_{s}") for s in range(BSUB)]
                ps_m = [psum.tile([128, OC], F32, tag="ps",
                                  name=f"psm{bq}_{s}") for s in range(BSUB)]
                # wave 0 is fold-paced: kt-outer lets all banks progress as
                # kt-halves land. Later waves run on resident u,v: bsub-outer
                # finishes each bank early so its assembly + out-DMA overlap
                # the remaining matmuls instead of piling up at wave end.
                if bq == 0:
                    order = [(kt, s) for kt in range(KT) for s in range(BSUB)]
                else:
                    order = [(kt, s) for s in range(BSUB) for kt in range(KT)]
                for kt, s in order:
                    st, sp = (kt == 0), (kt == KT - 1)
                    b0 = s * 128
                    lv = v[:, bq, kt, b0:b0 + 128]
                    lu = u[:, bq, kt, b0:b0 + 128]
                    nc.tensor.matmul(ps_m[s][:], lv, wt[:, kt, OC:],
                                     start=st, stop=False,
                                     skip_group_check=True)
                    nc.tensor.matmul(ps_p[s][:], lu, wt[:, kt, :OC],
                                     start=st, stop=sp)
                    # o=512 column rides on the sin bank's col 0
                    nc.tensor.matmul(ps_m[s][:, 0:1], lu, gt[:, 0:1],
                                     start=False, stop=sp,
                                     skip_group_check=True)
                for s in range(BSUB):
                    b0 = bq * QW + s * 128
                    # drain both banks via ACT (fast, idle) so PSUM frees
                    # for the next wave without waiting on the busy DVE
                    ec = ecp.tile([128, OC], F32, tag="ec", name=f"ec{bq}_{s}")
                    nc.scalar.copy(ec[:], ps_p[s][:])
                    mc = ecp.tile([128, OC], F32, tag="mc", name=f"mc{bq}_{s}")
                    nc.scalar.copy(mc[:], ps_m[s][:])
                    ot = outp.tile([128, 2, OC], BF16, tag="out",
                                   name=f"ot{bq}_{s}")
                    nc.vector.tensor_add(out=ot[:, 0], in0=ec[:],
                                         in1=mc[:])
                    nc.vector.tensor_sub(out=ot[:, 1], in0=ec[:],
                                         in1=mc[:])
                    nc.sync.dma_start(out_ap[b0:b0 + 128, :], ot[:])

    nc.compile()
    return nc


def kernel(x, filters_real, filters_imag):
    global LAST_RESULTS
    x = np.asarray(x, dtype=np.float32)
    fr = np.asarray(filters_real, dtype=np.float32)
    fi = np.asarray(filters_imag, dtype=np.float32)

    # weights: C = (w_r+w_i)/2 = 0.02cos, S = (w_r-w_i)/2 = 0.02sin over
    # the first period, transposed to [t, o]; o = 0..511 plus the o=512
    # cos column served by g (and sin col 0, identically 0, zeroed).
    w_r = fr + fi                           # [O, F]
    w_i = fr - fi
    cfull = 0.5 * (w_r[:, :T] + w_i[:, :T])   # [O, T] = 0.02 cos
    sfull = 0.5 * (w_r[:, :T] - w_i[:, :T])   # [O, T] = 0.02 sin
    w_np = np.empty((T, 2 * OC), np.float32)
    w_np[:, :OC] = cfull[:OC].T
    w_np[:, OC:] = sfull[:OC].T
    w_np[:, OC] = 0.0
    w_np = w_np.astype(BF16NP)
    g_np = np.ascontiguousarray(cfull[OC, :128][:, None]).astype(BF16NP)

    if "nc" not in _CACHE:
        _CACHE["nc"] = _build()
    nc = _CACHE["nc"]

    xbf = x.astype(BF16NP)                  # [B, 2, F]
    from concurrent.futures import ThreadPoolExecutor

    def _shard(c):
        xs = xbf[c * BS:(c + 1) * BS]       # [2048, 2, 2049]
        xt = np.empty((BQ, 4, 128, KT, QW), BF16NP)
        for ch in range(2):
            xct = np.ascontiguousarray(xs[:, ch, :2 * T].T)  # [2048t, 2048b]
            # [kt, p, bq, bs] -> [bq, p, kt, bs]
            lo = xct[:T].reshape(KT, 128, BQ, QW).transpose(2, 1, 0, 3)
            hi = xct[T:].reshape(KT, 128, BQ, QW).transpose(2, 1, 0, 3)
            xt[:, 2 * ch] = lo
            xt[:, 2 * ch + 1] = hi
        wrap = np.ascontiguousarray(xs[:, :, 2 * T].T)       # [2, 2048]
        return xt.reshape(BQ * 4 * 128, KT * QW), wrap

    with ThreadPoolExecutor(NCORES) as ex:
        shards = list(ex.map(_shard, range(NCORES)))
    in_maps = [{"xT": shards[c][0], "wrap": shards[c][1],
                "w": w_np, "g": g_np} for c in range(NCORES)]

    import os
    trace = bool(os.environ.get("BASS_TRACE"))
    if trace:
        try:
            import antenv.axon_hooks  # noqa: F401  (shim from test.py)
        except ImportError:
            trace = False
            os.environ["BASS_NEVER_TRACE"] = "1"
    res = run_bass_kernel_spmd(nc, in_maps, list(range(NCORES)), trace=trace)
    LAST_RESULTS = res

    out = np.empty((B, O), np.float32)

    def _gather(c):
        sc = np.asarray(res.results[c]["out"]).astype(np.float32)
        s1, s2 = sc[:, :OC], sc[:, OC:]
        oc = out[c * BS:(c + 1) * BS]
        oc[:, 0] = 0.5 * (s1[:, 0] + s2[:, 0])
        oc[:, 1:OC] = s1[:, 1:OC]
        oc[:, OC] = 0.5 * (s1[:, 0] - s2[:, 0])
        oc[:, OC + 1:] = s2[:, OC - 1:0:-1]

    with ThreadPoolExecutor(NCORES) as ex:
        list(ex.map(_gather, range(NCORES)))
    return out
